# revision 55
# baseline (speedup 1.0000x reference)
"""Trainium2 Bass kernel for nn_AttentionLayer (pooling attention).

Computes, for each batch b and head i:
    own  = inputs[b,i,:] @ W1_own[i] + b1[i]          # [64]
    ev   = inputs[b,j,:] @ W1_ev[i]                   # [j,64]
    h    = relu(own + ev)                             # [j,64]
    s    = h @ W2[i]                                  # [j]
    w    = softmax_j(s)
    out[b,i] = sum_j w[j] * inputs[b,j]

Key identity: max(ev, -(own+b1)) = relu(ev+own+b1) - (own+b1); the
correction is constant in j, so softmax is unchanged — no separate
relu pass needed.

Design (169us -> ~127us vs the all-bf16 v1):
  * ev contraction is hybrid fp8/bf16: d-tiles 0..3 run as DoubleRow
    fp8 matmuls (K=256 each, ~1.5x PE throughput), d-tiles 4..5 stay
    bf16.  W1e/W1o/b1 are pre-scaled by 1024 (lossless power of two)
    so fp8 weights sit in TRN e4m3's +-240 range; W2 is scaled by
    1/1024 so the softmax temperature is unchanged.  own runs fully
    fp8 DoubleRow — it only feeds the max threshold, whose error
    contribution is kink-limited.  Measured rel err 1.74e-2 < 2e-2.
  * bf16 k-tiles run FIRST within each (chunk, head-pair): an
    LDWEIGHTS can overlap a normal matmul's stream but not a DoubleRow
    one, so leading with bf16 hides the first DR weight load.
  * chunks are processed in PAIRS sharing every weight load (DR
    LDWEIGHTS cannot be pulled ahead past a DR matmul); the last two
    chunks stay single so the softmax/pool drain tail stays short,
    and the final chunk's scores+softmax run in column halves so its
    pooling starts while the second half is still computing.
  * own (j-major fp8 xt2, N=512 DoubleRow matmuls) rides chunk pair
    0 with a one-tile lag so its weight/input DMAs never stall the
    in-order PE queue.
  * DMA throughput is gated by per-partition descriptor size, not HBM
    bandwidth: each ev chunk ships as ONE packed byte tensor (4KB
    contiguous per partition, fp8 part then bf16 part, bitcast into
    views on SBUF); weights ship whole; only the first-consumed pieces
    are split smaller to beat the slow (~10us) DMA ramp at startup.
    Rings: sync = ev chunks + xn + outputs, scalar = ev weights + xt2
    + output casts, gpsimd = own weights.
  * pooling uses a mask-built block-diagonal weight matrix (8 batches
    x 16 evidences per 128-row block); PE transposes with a 16x16
    identity turn softmax rows into pooling weight columns.

Sharding: data-parallel over batch across 8 NeuronCores (256
batches/core).  All parameters replicated; no collectives.

Self-contained: hardcodes shapes; only needs /opt/trn_rl_repo on
sys.path.
"""

import os
import sys
from contextlib import ExitStack

import numpy as np

if "/opt/trn_rl_repo" not in sys.path:
    sys.path.insert(0, "/opt/trn_rl_repo")
os.environ.setdefault("MYCRO_LOCAL_CACHE", "1")

import ml_dtypes  # noqa: E402

import concourse.bass as bass  # noqa: E402
import concourse.mybir as mybir  # noqa: E402
import concourse.tile as tile  # noqa: E402
from concourse import bacc  # noqa: E402
from concourse import bass_utils  # noqa: E402

# Problem shapes (hardcoded per spec)
B, NINS, D, H = 2048, 16, 768, 64
NCORES = 8
BC = B // NCORES          # 256 batches per core
R = BC * NINS             # 4096 rows (b,j) per core
KT = D // 128             # 6 contraction k-tiles
KP8 = 2                   # fp8 DoubleRow k-pairs (d-tiles 0..3)
KB16 = KT - 2 * KP8       # trailing bf16 k-tiles (d-tiles 4..5)
MT = NINS // 2            # 8 m-tiles of (il,h): tile t holds heads 2t, 2t+1
NCH = 8                   # column chunks per core
CHUNK = R // NCH          # 512 (b,j) columns per chunk
CB = CHUNK // NINS        # 32 batches per chunk
NBLK = R // 128           # 32 row-blocks per core
WSCALE = 1024.0           # power-of-two pre-scale for W1/b1 (W2 /= it)

BF = mybir.dt.bfloat16
F32 = mybir.dt.float32
FP8 = mybir.dt.float8e4
BF_NP = ml_dtypes.bfloat16
FP8_NP = ml_dtypes.float8_e4m3

_CACHED_NC = None
LAST_RESULTS = None


def build_nc():
    nc = bacc.Bacc("TRN2", target_bir_lowering=False, debug=False,
                   num_devices=NCORES)

    # b-major transposes of x for the ev matmuls, chunk-major, PACKED
    # as bytes (fp8 DoubleRow part then bf16 part) so each chunk is one
    # DMA with a 4KB contiguous per-partition segment — small
    # descriptors gate DMA throughput, not HBM bandwidth.
    U8 = mybir.dt.uint8
    xtc_d = nc.dram_tensor("xtc", [128, NCH, 4096], U8,
                           kind="ExternalInput").ap()
    # j-major transpose for own, (kp, ko)-interleaved for DoubleRow
    # (own only feeds the max threshold, whose error contribution is
    # kink-limited, so fully-fp8 own is safe: measured rel 1.73e-2)
    xt2_d = nc.dram_tensor("xt2", [128, MT, KT // 2, 2, CHUNK], FP8,
                           kind="ExternalInput").ap()
    xn_d = nc.dram_tensor("xn", [128, NBLK, D], BF,
                          kind="ExternalInput").ap()
    w1e8_d = nc.dram_tensor("w1e8", [128, KP8, 2, NINS * H], FP8,
                            kind="ExternalInput").ap()
    w1eb_d = nc.dram_tensor("w1eb", [128, KB16, NINS * H], BF,
                            kind="ExternalInput").ap()
    w1o_d = nc.dram_tensor("w1o8", [128, KT // 2, 2, NINS * H], FP8,
                           kind="ExternalInput").ap()
    w2b_d = nc.dram_tensor("w2b", [128, MT, 128], BF,
                           kind="ExternalInput").ap()
    b1n_d = nc.dram_tensor("b1n", [128, MT], F32, kind="ExternalInput").ap()
    msk_d = nc.dram_tensor("msk", [128, 128], BF, kind="ExternalInput").ap()
    idn_d = nc.dram_tensor("idn", [16, 16], BF, kind="ExternalInput").ap()
    out_d = nc.dram_tensor("out", [128, NBLK, D], BF,
                           kind="ExternalOutput").ap()

    with tile.TileContext(nc) as tc, ExitStack() as ctx:
        const = ctx.enter_context(tc.tile_pool(name="const", bufs=1))
        xtp = ctx.enter_context(tc.tile_pool(name="xtp", bufs=4))
        xt2p = ctx.enter_context(tc.tile_pool(name="xt2p", bufs=4))
        xnp = ctx.enter_context(tc.tile_pool(name="xnp", bufs=6))
        ownsb = ctx.enter_context(tc.tile_pool(name="ownsb", bufs=1))
        hp = ctx.enter_context(tc.tile_pool(name="hp", bufs=18))
        sm = ctx.enter_context(tc.tile_pool(name="sm", bufs=2))
        bdp = ctx.enter_context(tc.tile_pool(name="bdp", bufs=3))
        outp = ctx.enter_context(tc.tile_pool(name="outp", bufs=4))
        # PSUM (8 banks): ev(+wgtT borrow) 5 + scores 1 + pool/own 2.
        # own runs only during pair 0, pooling only starts after pair 0,
        # so they share the plps ring.  scores need only 1 bank: the
        # exp that frees it runs ~17us before the next group's scores.
        # 5 ev banks push the ring-WAR at a pair boundary back to h(t5),
        # which is long done — no PE stall waiting on the vector queue.
        evps = ctx.enter_context(tc.tile_pool(name="evps", bufs=5,
                                              space="PSUM"))
        scps = ctx.enter_context(tc.tile_pool(name="scps", bufs=1,
                                              space="PSUM"))
        plps = ctx.enter_context(tc.tile_pool(name="plps", bufs=2,
                                              space="PSUM"))

        # --- constants.  Whole-tensor DMAs: small per-partition
        # descriptors (not HBM bandwidth) gate DMA throughput, so every
        # transfer moves large contiguous per-partition segments.
        b1n_sb = const.tile([128, MT], F32, tag="b1n")
        nc.scalar.dma_start(b1n_sb[:], b1n_d[:])
        w1e8_sb = const.tile([128, KP8, 2, NINS * H], FP8, tag="w1e8")
        w1eb_sb = const.tile([128, KB16, NINS * H], BF, tag="w1eb")
        w1o_sb = const.tile([128, KT // 2, 2, NINS * H], FP8, tag="w1o")
        nc.gpsimd.dma_start(w1o_sb[:, 0], w1o_d[:, 0])
        nc.gpsimd.dma_start(w1o_sb[:, 1:], w1o_d[:, 1:])
        w2b_sb = const.tile([128, MT, 128], BF, tag="w2b")
        msk_sb = const.tile([128, 128], BF, tag="msk")
        idn_sb = const.tile([16, 16], BF, tag="idn")

        xt_tiles = {}   # c -> (fp8 view, bf16 view)
        xt2_tiles = {}  # t -> AP view into a pair tile
        xn_tiles = {}

        def dma_xt(c):
            if c >= NCH:
                return
            tc_ = xtp.tile([128, 4096], mybir.dt.uint8, tag="xtc",
                           name="xtct")
            nc.sync.dma_start(tc_[:], xtc_d[:, c])
            t8 = tc_[:, :2048].bitcast(FP8).rearrange(
                "p (kp ko n) -> p kp ko n", kp=KP8, ko=2)
            tb = tc_[:, 2048:].bitcast(BF).rearrange(
                "p (kb n) -> p kb n", kb=KB16)
            xt_tiles[c] = (t8, tb)

        def dma_xt2_pair(tp_, engine):
            # two j-major tiles per DMA: 6KB per-partition segments
            pt = xt2p.tile([128, 2, KT // 2, 2, CHUNK], FP8, tag="xt2",
                           name="xt2t")
            engine.dma_start(pt[:], xt2_d[:, 2 * tp_:2 * tp_ + 2])
            xt2_tiles[2 * tp_] = pt[:, 0]
            xt2_tiles[2 * tp_ + 1] = pt[:, 1]

        def dma_xn(c):
            if c >= NCH:
                return
            t_ = xnp.tile([128, 4, D], BF, tag="xn", name="xnt")
            nc.sync.dma_start(t_[:], xn_d[:, c * 4:(c + 1) * 4, :])
            xn_tiles[c] = t_

        # startup DMA order (first consumed first per ring).  The DMA
        # subsystem ramps slowly in the first ~10us, so the pieces the
        # PE needs first ship as smaller DMAs; everything later ships
        # whole (small per-partition descriptors throttle throughput).
        tc0 = xtp.tile([128, 4096], mybir.dt.uint8, tag="xtc", name="xtct")
        # first weight piece rides the sync ring's front: scalar's front
        # is taxed by b1n's tiny-descriptor DMA, sync starts earliest
        nc.sync.dma_start(w1eb_sb[:, 0, :512], w1eb_d[:, 0, :512])
        nc.sync.dma_start(tc0[:, 2048:3072], xtc_d[:, 0, 2048:3072])
        nc.sync.dma_start(tc0[:, 3072:4096], xtc_d[:, 0, 3072:4096])
        nc.sync.dma_start(tc0[:, :1024], xtc_d[:, 0, :1024])
        nc.sync.dma_start(tc0[:, 1024:2048], xtc_d[:, 0, 1024:2048])
        xt_tiles[0] = (
            tc0[:, :2048].bitcast(FP8).rearrange(
                "p (kp ko n) -> p kp ko n", kp=KP8, ko=2),
            tc0[:, 2048:].bitcast(BF).rearrange(
                "p (kb n) -> p kb n", kb=KB16))
        nc.scalar.dma_start(w1eb_sb[:, 0, 512:], w1eb_d[:, 0, 512:])
        nc.scalar.dma_start(w1eb_sb[:, 1], w1eb_d[:, 1])
        nc.scalar.dma_start(w1e8_sb[:, 0, :, :512], w1e8_d[:, 0, :, :512])
        nc.scalar.dma_start(w1e8_sb[:, 0, :, 512:], w1e8_d[:, 0, :, 512:])
        nc.scalar.dma_start(w1e8_sb[:, 1], w1e8_d[:, 1])
        dma_xt(1)
        dma_xt2_pair(0, nc.scalar)
        dma_xt2_pair(1, nc.scalar)
        dma_xt(2)
        dma_xt2_pair(2, nc.sync)
        nc.scalar.dma_start(w2b_sb[:], w2b_d[:])
        nc.scalar.dma_start(idn_sb[:], idn_d[:])
        nc.scalar.dma_start(msk_sb[:], msk_d[:])
        dma_xt(3)
        dma_xt2_pair(3, nc.sync)
        dma_xn(0)
        dma_xn(1)

        # --- own: ownneg128[(il,h), t, b] = -(own'[b,2t+il,h] + b1'[2t+il,h])
        # (primes = x WSCALE).  One N=512 matmul per (t, k) on the
        # j-major layout; interleaved into pair 0's ev stream, one
        # tile behind so its DMAs never stall the PE queue.
        own128 = ownsb.tile([128, MT, BC], BF, tag="own")
        own_ps = {}

        def own_mm(t, kp):
            if t < 0 or t >= MT or kp >= KT // 2:
                return
            if kp == 0:
                own_ps[t] = plps.tile([128, 2, BC], F32, tag="pp",
                                      name="ownp")
            nc.tensor.matmul(
                own_ps[t][:], lhsT=w1o_sb[:, kp, :, t * 128:(t + 1) * 128],
                rhs=xt2_tiles[t][:, kp, :, :],
                start=(kp == 0), stop=(kp == KT // 2 - 1),
                perf_mode=mybir.MatmulPerfMode.DoubleRow,
                skip_group_check=True,
            )

        def own_retire(t):
            # on scalar (activation): out = Copy(in * -1 + b1n) — the
            # vector engine is the busy one during pair 0
            if t >= MT:
                return
            ops = own_ps.pop(t)
            for il in range(2):
                nc.scalar.activation(
                    own128[il * H:(il + 1) * H, t, :],
                    ops[il * H:(il + 1) * H, il, :],
                    mybir.ActivationFunctionType.Identity,
                    bias=b1n_sb[il * H:(il + 1) * H, t, None],
                    scale=-1.0)

        def softmax_cols(scp, wgt, hf):
            # softmax of one 256-column half of the last chunk's scores
            bs = slice(hf * (CB // 2), (hf + 1) * (CB // 2))
            cols = slice(hf * 256, (hf + 1) * 256)
            ex = sm.tile([NINS, CB, NINS], F32, tag="ex")
            ssum = sm.tile([NINS, CB], F32, tag="ssum")
            rinv = sm.tile([NINS, CB], F32, tag="rinv")
            nc.scalar.activation(
                ex[:, bs, :],
                scp[:NINS, :].rearrange("p (b j) -> p b j", j=NINS)[:, bs, :],
                mybir.ActivationFunctionType.Exp)
            nc.vector.tensor_reduce(ssum[:, bs], ex[:, bs, :],
                                    axis=mybir.AxisListType.X,
                                    op=mybir.AluOpType.add)
            nc.vector.reciprocal(rinv[:, bs], ssum[:, bs])
            nc.vector.tensor_tensor(
                wgt.rearrange("p (b j) -> p b j", j=NINS)[:, bs, :],
                ex[:, bs, :],
                rinv[:, bs, None].to_broadcast([NINS, CB // 2, NINS]),
                mybir.AluOpType.mult)

        def do_softmax(scp, halves=1):
            # scores are O(3); safe to exp without max subtraction.
            # only 16 of 128 rows hold scores; the rest are exact zeros
            # (the col-tiled matmuls write zeros there) so everything
            # stays finite and the sel-transpose drops them.
            # halves=2 (last chunk): process column halves separately so
            # pooling of the first half starts while the second runs.
            ex = sm.tile([NINS, CB, NINS], F32, tag="ex")
            ssum = sm.tile([NINS, CB], F32, tag="ssum")
            rinv = sm.tile([NINS, CB], F32, tag="rinv")
            wgt = sm.tile([NINS, CHUNK], BF, tag="wgt")
            hb = CB // halves
            for hf in range(halves):
                bs = slice(hf * hb, (hf + 1) * hb)
                nc.scalar.activation(
                    ex[:, bs, :],
                    scp.rearrange("p (b j) -> p b j", j=NINS)[:, bs, :],
                    mybir.ActivationFunctionType.Exp)
                nc.vector.tensor_reduce(ssum[:, bs], ex[:, bs, :],
                                        axis=mybir.AxisListType.X,
                                        op=mybir.AluOpType.add)
                nc.vector.reciprocal(rinv[:, bs], ssum[:, bs])
                nc.vector.tensor_tensor(
                    wgt.rearrange("p (b j) -> p b j", j=NINS)[:, bs, :],
                    ex[:, bs, :],
                    rinv[:, bs, None].to_broadcast([NINS, hb, NINS]),
                    mybir.AluOpType.mult)
            return wgt

        def emit_wgtT(wgt, rts, tpf=None):
            # borrow one evps ring buffer; bitcast a bf16 view for the
            # transpose outputs ([128, 4, 16] bf16 = 128 f32 bytes).
            # rts selects which 128-column blocks to transpose, so the
            # last chunk can transpose each softmax half as it lands.
            if tpf is None:
                tpf = evps.tile([128, CHUNK], F32, tag="ev")
            tp = tpf[:, :32].bitcast(BF).rearrange("p (r i) -> p r i", i=NINS)
            for rt in rts:
                nc.tensor.transpose(tp[:, rt, :],
                                    wgt[:, rt * 128:(rt + 1) * 128],
                                    idn_sb[:])
            return tpf, tp

        def emit_pool_rt(c, tp, rt):
            bd = bdp.tile([128, 8, NINS], BF, tag="bd")
            nc.vector.tensor_tensor(
                bd[:], tp[:, rt, None, :].to_broadcast([128, 8, NINS]),
                msk_sb.rearrange("p (g i) -> p g i", i=NINS),
                mybir.AluOpType.mult)
            bdf = bd.rearrange("p g i -> p (g i)")
            pp0 = plps.tile([128, 384], F32, tag="pp")
            pp1 = plps.tile([128, 384], F32, tag="pp")
            nc.tensor.matmul(pp0[:], lhsT=bdf, rhs=xn_tiles[c][:, rt, :384],
                             start=True, stop=True, skip_group_check=True)
            nc.tensor.matmul(pp1[:], lhsT=bdf, rhs=xn_tiles[c][:, rt, 384:],
                             start=True, stop=True, skip_group_check=True)
            return pp0, pp1

        def emit_out_rt(c, rt, pp0, pp1, drain=False):
            osb = outp.tile([128, D], BF, tag="osb")
            if drain:
                # tail: consecutive rt pieces reuse the same two PSUM
                # banks, so their copies gate the next pool matmuls —
                # alternate the copy engine per rt so they overlap,
                # and alternate output DMA rings
                if rt % 2:
                    nc.vector.tensor_copy(osb[:, :384], pp0[:])
                    nc.vector.tensor_copy(osb[:, 384:], pp1[:])
                    nc.sync.dma_start(out_d[:, c * 4 + rt, :], osb[:])
                else:
                    nc.scalar.copy(osb[:, :384], pp0[:])
                    nc.scalar.copy(osb[:, 384:], pp1[:])
                    nc.gpsimd.dma_start(out_d[:, c * 4 + rt, :], osb[:])
            else:
                nc.scalar.copy(osb[:, :384], pp0[:])
                nc.scalar.copy(osb[:, 384:], pp1[:])
                nc.sync.dma_start(out_d[:, c * 4 + rt, :], osb[:])

        pqueue = []  # chunks awaiting pooling: {c, wgt, tp, piece}

        SEQ_NORM = [("tp", (0, 1, 2, 3)), ("rt", 0), ("rt", 1),
                    ("rt", 2), ("rt", 3)]
        SEQ_SPLIT = [("tp", (0, 1)), ("rt", 0), ("rt", 1),
                     ("tp", (2, 3)), ("rt", 2), ("rt", 3)]

        def pool_advance(drain=False):
            """Emit the next pooling piece (wgtT, then rt blocks)."""
            if not pqueue:
                return
            st = pqueue[0]
            seq = SEQ_SPLIT if st.get("split") else SEQ_NORM
            kind, arg = seq[st["piece"]]
            if kind == "tp":
                st["tpf"], st["tp"] = emit_wgtT(st["wgt"], arg,
                                                st.get("tpf"))
            else:
                ppa, ppb = emit_pool_rt(st["c"], st["tp"], arg)
                emit_out_rt(st["c"], arg, ppa, ppb, drain=drain)
            st["piece"] += 1
            if st["piece"] == len(seq):
                pqueue.pop(0)

        def emit_scores(hts):
            scp = scps.tile([128, CHUNK], F32, tag="scp")
            for t in range(MT):
                nc.tensor.matmul(
                    scp[:], lhsT=w2b_sb[:, t, :], rhs=hts[t],
                    start=(t == 0), stop=(t == MT - 1),
                    skip_group_check=True,  # rows 16+ all-zero
                )
            return scp

        def ev_mms(cs, t, evs, with_own):
            """ev for tile t of every chunk in cs (shared weight loads);
            pair 0 interleaves own tile t-1 between the k-groups.
            bf16 k-tiles go FIRST: an LDWEIGHTS can overlap a normal
            matmul's stream but not a DoubleRow one, so leading with
            bf16 hides the first DR weight load under the bf16 MMs."""
            ts = [xt_tiles[c] for c in cs]
            ok = iter(range(KT))
            for kb in range(KB16):
                for ci, c in enumerate(cs):
                    nc.tensor.matmul(
                        evs[ci][:],
                        lhsT=w1eb_sb[:, kb, t * 128:(t + 1) * 128],
                        rhs=ts[ci][1][:, kb, :],
                        start=(kb == 0), stop=False,
                        skip_group_check=True,
                    )
                if with_own:
                    own_mm(t - 1, next(ok))
            for kp in range(KP8):
                for ci, c in enumerate(cs):
                    nc.tensor.matmul(
                        evs[ci][:],
                        lhsT=w1e8_sb[:, kp, :, t * 128:(t + 1) * 128],
                        rhs=ts[ci][0][:, kp, :, :],
                        start=False, stop=(kp == KP8 - 1),
                        perf_mode=mybir.MatmulPerfMode.DoubleRow,
                        skip_group_check=True,
                    )
                if with_own:
                    own_mm(t - 1, next(ok))
                    own_mm(t - 1, next(ok))

        def emit_h(c, t, evp):
            h_t = hp.tile([128, CB, NINS], BF, tag="h")
            nc.vector.tensor_tensor(
                h_t[:], evp.rearrange("p (b j) -> p b j", j=NINS),
                own128[:, t, c * CB:(c + 1) * CB, None]
                .to_broadcast([128, CB, NINS]),
                mybir.AluOpType.max)
            return h_t.rearrange("p b j -> p (b j)")

        def emit_group(cs, with_own=False, last=False):
            """Process chunks cs (1 or 2) through ev/h/scores/softmax,
            while draining the pooling of previous chunks."""
            hts = {c: [] for c in cs}
            lag = 1 if with_own else 0
            evps_t = {}
            for t in range(MT + lag):
                if t < MT:
                    evps_t[t] = [evps.tile([128, CHUNK], F32, tag="ev",
                                           name="evt")
                                 for _ in cs]
                    ev_mms(cs, t, evps_t[t], with_own)
                elif with_own:
                    # own tile 7 has no ev matmuls left to ride on
                    for k in range(KT):
                        own_mm(MT - 1, k)
                th = t - lag
                if 0 <= th < MT:
                    if with_own:
                        own_retire(th)
                    for ci, c in enumerate(cs):
                        hts[c].append(emit_h(c, th, evps_t[th][ci]))
                        # skip the first slot: gives the previous
                        # chunk's softmax time to finish before wgtT
                        if th >= 1:
                            pool_advance()
                    del evps_t[th]
                if t == 4:
                    for c in cs:
                        dma_xn(c + 2)
            if last:
                # final chunk: scores and softmax in column halves so
                # the pooling drain starts while half B is still going
                c = cs[0]
                scp = scps.tile([128, CHUNK], F32, tag="scp")
                wgt = sm.tile([NINS, CHUNK], BF, tag="wgt")
                for hf in range(2):
                    cols = slice(hf * 256, (hf + 1) * 256)
                    for t in range(MT):
                        nc.tensor.matmul(
                            scp[:, cols], lhsT=w2b_sb[:, t, :],
                            rhs=hts[c][t][:, cols],
                            start=(t == 0), stop=(t == MT - 1),
                            skip_group_check=True)
                    softmax_cols(scp, wgt, hf)
                pqueue.append({"c": c, "wgt": wgt, "tp": None,
                               "piece": 0, "split": True})
                return
            scps_c = [emit_scores(hts[c]) for c in cs]
            # pooling not covered by the h slots drains AFTER scores:
            # it has slack, while scores gate the softmax/pool chain
            while pqueue:
                pool_advance()
            for c in cs:
                dma_xt(c + 4)
            for ci, c in enumerate(cs):
                pqueue.append({"c": c,
                               "wgt": do_softmax(scps_c[ci][:NINS, :]),
                               "tp": None, "piece": 0})

        emit_group([0, 1], with_own=True)
        emit_group([2, 3])
        emit_group([4, 5])
        emit_group([6])
        emit_group([7], last=True)

        # drain the last chunk's pooling
        while pqueue:
            pool_advance(drain=True)

    nc.compile()
    return nc


def host_prep(W1, b1, W2):
    """Build the replicated parameter tensors (numpy)."""
    W1 = np.asarray(W1, dtype=np.float32)
    b1 = np.asarray(b1, dtype=np.float32)
    W2 = np.asarray(W2, dtype=np.float32)
    W1o, W1e = W1[:, :D, :] * WSCALE, W1[:, D:, :] * WSCALE

    def to_cols(w):  # [16, 768, 64] -> [768, 1024] (cols i*64+h)
        return np.ascontiguousarray(
            w.transpose(1, 0, 2).reshape(D, NINS * H))

    we = to_cols(W1e)   # [768, 1024], pre-scaled
    # fp8 DoubleRow part: d-tiles 0..3 -> [128, KP8, 2, 1024]
    w1e8 = np.ascontiguousarray(
        we[:512].reshape(KP8, 2, 128, NINS * H)
        .transpose(2, 0, 1, 3)).astype(FP8_NP)
    # bf16 part: d-tiles 4..5 -> [128, KB16, 1024]
    w1eb = np.ascontiguousarray(
        we[512:].reshape(KB16, 128, NINS * H)
        .transpose(1, 0, 2)).astype(BF_NP)
    w1o8 = np.ascontiguousarray(
        to_cols(W1o).reshape(KT // 2, 2, 128, NINS * H)
        .transpose(2, 0, 1, 3)).astype(FP8_NP)

    w2b = np.zeros((128, MT, 128), dtype=np.float32)
    b1n = np.zeros((128, MT), dtype=np.float32)
    for t in range(MT):
        for il in range(2):
            i = 2 * t + il
            w2b[il * H:(il + 1) * H, t, i] = W2[i] / WSCALE
            b1n[il * H:(il + 1) * H, t] = -b1[i] * WSCALE
    p = np.arange(128)
    msk = (p[:, None] // NINS == p[None, :] // NINS).astype(BF_NP)
    idn = np.eye(16, dtype=np.float32).astype(BF_NP)
    return dict(w1e8=w1e8, w1eb=w1eb, w1o8=w1o8, w2b=w2b.astype(BF_NP),
                b1n=b1n, msk=msk, idn=idn)


def get_nc():
    global _CACHED_NC
    if _CACHED_NC is None:
        _CACHED_NC = build_nc()
    return _CACHED_NC


def make_in_maps(inputs, W1, b1, W2):
    consts = host_prep(W1, b1, W2)
    inputs = np.asarray(inputs, dtype=np.float32)
    in_maps = []
    for core in range(NCORES):
        shard = np.ascontiguousarray(
            inputs[core * BC:(core + 1) * BC].reshape(R, D))
        m = dict(consts)
        # natural rows, blocked: xn[p, blk, :] = x[blk*128+p, :]
        m["xn"] = np.ascontiguousarray(
            shard.reshape(NBLK, 128, D).transpose(1, 0, 2)).astype(BF_NP)
        st = shard.T  # [768, 4096]
        # ev chunk data packed as bytes: fp8 DoubleRow part (d-tiles
        # 0..3, (kp, ko) interleave) then bf16 part (d-tiles 4..5)
        xt8 = np.ascontiguousarray(
            st[:512].reshape(KP8, 2, 128, NCH, CHUNK)
            .transpose(2, 3, 0, 1, 4)).astype(FP8_NP)
        xtb = np.ascontiguousarray(
            st[512:].reshape(KB16, 128, NCH, CHUNK)
            .transpose(1, 2, 0, 3)).astype(BF_NP)
        m["xtc"] = np.concatenate(
            [xt8.reshape(128, NCH, 2048).view(np.uint8),
             xtb.reshape(128, NCH, 1024).view(np.uint8)], axis=2)
        # j-major transpose for own: rows (j, b); xt2[p, t, k, col]
        x2 = shard.reshape(BC, NINS, D).transpose(1, 0, 2).reshape(R, D)
        m["xt2"] = np.ascontiguousarray(
            x2.T.reshape(KT // 2, 2, 128, MT, CHUNK)
            .transpose(2, 3, 0, 1, 4)).astype(FP8_NP)
        in_maps.append(m)
    return in_maps


def kernel(inputs, W1, b1, W2, b2, trace=False):
    """Full-input entry point: shards over 8 cores, returns full output."""
    global LAST_RESULTS
    nc = get_nc()
    in_maps = make_in_maps(inputs, W1, b1, W2)
    res = bass_utils.run_bass_kernel_spmd(
        nc, in_maps, core_ids=list(range(NCORES)), trace=trace)
    LAST_RESULTS = res
    out = np.concatenate(
        [np.asarray(r["out"]).astype(np.float32).transpose(1, 0, 2)
         .reshape(BC, NINS, D)
         for r in res.results],
        axis=0)
    return out


if __name__ == "__main__":
    if "--build" in sys.argv:
        get_nc()
        print("build OK")


# revision 56
# speedup vs baseline: 1.0065x; 1.0065x over previous
"""Trainium2 Bass kernel for nn_AttentionLayer (pooling attention).

Computes, for each batch b and head i:
    own  = inputs[b,i,:] @ W1_own[i] + b1[i]          # [64]
    ev   = inputs[b,j,:] @ W1_ev[i]                   # [j,64]
    h    = relu(own + ev)                             # [j,64]
    s    = h @ W2[i]                                  # [j]
    w    = softmax_j(s)
    out[b,i] = sum_j w[j] * inputs[b,j]

Key identity: max(ev, -(own+b1)) = relu(ev+own+b1) - (own+b1); the
correction is constant in j, so softmax is unchanged — no separate
relu pass needed.

Design (169us -> ~127us vs the all-bf16 v1):
  * ev contraction is hybrid fp8/bf16: d-tiles 0..3 run as DoubleRow
    fp8 matmuls (K=256 each, ~1.5x PE throughput), d-tiles 4..5 stay
    bf16.  W1e/W1o/b1 are pre-scaled by 1024 (lossless power of two)
    so fp8 weights sit in TRN e4m3's +-240 range; W2 is scaled by
    1/1024 so the softmax temperature is unchanged.  own runs fully
    fp8 DoubleRow — it only feeds the max threshold, whose error
    contribution is kink-limited.  Measured rel err 1.74e-2 < 2e-2.
  * bf16 k-tiles run FIRST within each (chunk, head-pair): an
    LDWEIGHTS can overlap a normal matmul's stream but not a DoubleRow
    one, so leading with bf16 hides the first DR weight load.
  * chunks are processed in PAIRS sharing every weight load (DR
    LDWEIGHTS cannot be pulled ahead past a DR matmul); the last two
    chunks stay single so the softmax/pool drain tail stays short,
    and the final chunk's scores+softmax run in column halves so its
    pooling starts while the second half is still computing.
  * own (j-major fp8 xt2, N=512 DoubleRow matmuls) rides chunk pair
    0 with a one-tile lag so its weight/input DMAs never stall the
    in-order PE queue.
  * DMA throughput is gated by per-partition descriptor size, not HBM
    bandwidth: each ev chunk ships as ONE packed byte tensor (4KB
    contiguous per partition, fp8 part then bf16 part, bitcast into
    views on SBUF); weights ship whole; only the first-consumed pieces
    are split smaller to beat the slow (~10us) DMA ramp at startup.
    Rings: sync = ev chunks + xn + outputs, scalar = ev weights + xt2
    + output casts, gpsimd = own weights.
  * pooling uses a mask-built block-diagonal weight matrix (8 batches
    x 16 evidences per 128-row block); PE transposes with a 16x16
    identity turn softmax rows into pooling weight columns.

Sharding: data-parallel over batch across 8 NeuronCores (256
batches/core).  All parameters replicated; no collectives.

Self-contained: hardcodes shapes; only needs /opt/trn_rl_repo on
sys.path.
"""

import os
import sys
from contextlib import ExitStack

import numpy as np

if "/opt/trn_rl_repo" not in sys.path:
    sys.path.insert(0, "/opt/trn_rl_repo")
os.environ.setdefault("MYCRO_LOCAL_CACHE", "1")

import ml_dtypes  # noqa: E402

import concourse.bass as bass  # noqa: E402
import concourse.mybir as mybir  # noqa: E402
import concourse.tile as tile  # noqa: E402
from concourse import bacc  # noqa: E402
from concourse import bass_utils  # noqa: E402

# Problem shapes (hardcoded per spec)
B, NINS, D, H = 2048, 16, 768, 64
NCORES = 8
BC = B // NCORES          # 256 batches per core
R = BC * NINS             # 4096 rows (b,j) per core
KT = D // 128             # 6 contraction k-tiles
KP8 = 2                   # fp8 DoubleRow k-pairs (d-tiles 0..3)
KB16 = KT - 2 * KP8       # trailing bf16 k-tiles (d-tiles 4..5)
MT = NINS // 2            # 8 m-tiles of (il,h): tile t holds heads 2t, 2t+1
NCH = 8                   # column chunks per core
CHUNK = R // NCH          # 512 (b,j) columns per chunk
CB = CHUNK // NINS        # 32 batches per chunk
NBLK = R // 128           # 32 row-blocks per core
WSCALE = 1024.0           # power-of-two pre-scale for W1/b1 (W2 /= it)

BF = mybir.dt.bfloat16
F32 = mybir.dt.float32
FP8 = mybir.dt.float8e4
BF_NP = ml_dtypes.bfloat16
FP8_NP = ml_dtypes.float8_e4m3

_CACHED_NC = None
LAST_RESULTS = None


def build_nc():
    nc = bacc.Bacc("TRN2", target_bir_lowering=False, debug=False,
                   num_devices=NCORES)

    # b-major transposes of x for the ev matmuls, chunk-major, PACKED
    # as bytes (fp8 DoubleRow part then bf16 part) so each chunk is one
    # DMA with a 4KB contiguous per-partition segment — small
    # descriptors gate DMA throughput, not HBM bandwidth.
    U8 = mybir.dt.uint8
    xtc_d = nc.dram_tensor("xtc", [128, NCH, 4096], U8,
                           kind="ExternalInput").ap()
    # j-major transpose for own, (kp, ko)-interleaved for DoubleRow
    # (own only feeds the max threshold, whose error contribution is
    # kink-limited, so fully-fp8 own is safe: measured rel 1.73e-2)
    xt2_d = nc.dram_tensor("xt2", [128, MT, KT // 2, 2, CHUNK], FP8,
                           kind="ExternalInput").ap()
    xn_d = nc.dram_tensor("xn", [128, NBLK, D], BF,
                          kind="ExternalInput").ap()
    w1e8_d = nc.dram_tensor("w1e8", [128, KP8, 2, NINS * H], FP8,
                            kind="ExternalInput").ap()
    w1eb_d = nc.dram_tensor("w1eb", [128, KB16, NINS * H], BF,
                            kind="ExternalInput").ap()
    w1o_d = nc.dram_tensor("w1o8", [128, KT // 2, 2, NINS * H], FP8,
                           kind="ExternalInput").ap()
    w2b_d = nc.dram_tensor("w2b", [128, MT, 128], BF,
                           kind="ExternalInput").ap()
    b1n_d = nc.dram_tensor("b1n", [128, MT], F32, kind="ExternalInput").ap()
    msk_d = nc.dram_tensor("msk", [128, 128], BF, kind="ExternalInput").ap()
    idn_d = nc.dram_tensor("idn", [16, 16], BF, kind="ExternalInput").ap()
    out_d = nc.dram_tensor("out", [128, NBLK, D], BF,
                           kind="ExternalOutput").ap()

    with tile.TileContext(nc) as tc, ExitStack() as ctx:
        const = ctx.enter_context(tc.tile_pool(name="const", bufs=1))
        xtp = ctx.enter_context(tc.tile_pool(name="xtp", bufs=4))
        xt2p = ctx.enter_context(tc.tile_pool(name="xt2p", bufs=4))
        xnp = ctx.enter_context(tc.tile_pool(name="xnp", bufs=6))
        ownsb = ctx.enter_context(tc.tile_pool(name="ownsb", bufs=1))
        hp = ctx.enter_context(tc.tile_pool(name="hp", bufs=18))
        sm = ctx.enter_context(tc.tile_pool(name="sm", bufs=2))
        bdp = ctx.enter_context(tc.tile_pool(name="bdp", bufs=3))
        outp = ctx.enter_context(tc.tile_pool(name="outp", bufs=4))
        # PSUM (8 banks): ev(+wgtT borrow) 5 + scores 1 + pool/own 2.
        # own runs only during pair 0, pooling only starts after pair 0,
        # so they share the plps ring.  scores need only 1 bank: the
        # exp that frees it runs ~17us before the next group's scores.
        # 5 ev banks push the ring-WAR at a pair boundary back to h(t5),
        # which is long done — no PE stall waiting on the vector queue.
        evps = ctx.enter_context(tc.tile_pool(name="evps", bufs=5,
                                              space="PSUM"))
        scps = ctx.enter_context(tc.tile_pool(name="scps", bufs=1,
                                              space="PSUM"))
        plps = ctx.enter_context(tc.tile_pool(name="plps", bufs=2,
                                              space="PSUM"))

        # --- constants.  Whole-tensor DMAs: small per-partition
        # descriptors (not HBM bandwidth) gate DMA throughput, so every
        # transfer moves large contiguous per-partition segments.
        b1n_sb = const.tile([128, MT], F32, tag="b1n")
        nc.scalar.dma_start(b1n_sb[:], b1n_d[:])
        w1e8_sb = const.tile([128, KP8, 2, NINS * H], FP8, tag="w1e8")
        w1eb_sb = const.tile([128, KB16, NINS * H], BF, tag="w1eb")
        w1o_sb = const.tile([128, KT // 2, 2, NINS * H], FP8, tag="w1o")
        nc.gpsimd.dma_start(w1o_sb[:, 0], w1o_d[:, 0])
        nc.gpsimd.dma_start(w1o_sb[:, 1:], w1o_d[:, 1:])
        w2b_sb = const.tile([128, MT, 128], BF, tag="w2b")
        msk_sb = const.tile([128, 128], BF, tag="msk")
        idn_sb = const.tile([16, 16], BF, tag="idn")

        xt_tiles = {}   # c -> (fp8 view, bf16 view)
        xt2_tiles = {}  # t -> AP view into a pair tile
        xn_tiles = {}

        def dma_xt(c):
            if c >= NCH:
                return
            tc_ = xtp.tile([128, 4096], mybir.dt.uint8, tag="xtc",
                           name="xtct")
            nc.sync.dma_start(tc_[:], xtc_d[:, c])
            t8 = tc_[:, :2048].bitcast(FP8).rearrange(
                "p (kp ko n) -> p kp ko n", kp=KP8, ko=2)
            tb = tc_[:, 2048:].bitcast(BF).rearrange(
                "p (kb n) -> p kb n", kb=KB16)
            xt_tiles[c] = (t8, tb)

        def dma_xt2_pair(tp_, engine):
            # two j-major tiles per DMA: 6KB per-partition segments
            pt = xt2p.tile([128, 2, KT // 2, 2, CHUNK], FP8, tag="xt2",
                           name="xt2t")
            engine.dma_start(pt[:], xt2_d[:, 2 * tp_:2 * tp_ + 2])
            xt2_tiles[2 * tp_] = pt[:, 0]
            xt2_tiles[2 * tp_ + 1] = pt[:, 1]

        def dma_xn(c):
            if c >= NCH:
                return
            t_ = xnp.tile([128, 4, D], BF, tag="xn", name="xnt")
            nc.sync.dma_start(t_[:], xn_d[:, c * 4:(c + 1) * 4, :])
            xn_tiles[c] = t_

        # startup DMA order (first consumed first per ring).  The DMA
        # subsystem ramps slowly in the first ~10us, so the pieces the
        # PE needs first ship as smaller DMAs; everything later ships
        # whole (small per-partition descriptors throttle throughput).
        tc0 = xtp.tile([128, 4096], mybir.dt.uint8, tag="xtc", name="xtct")
        nc.sync.dma_start(tc0[:, 2048:3072], xtc_d[:, 0, 2048:3072])
        nc.sync.dma_start(tc0[:, 3072:4096], xtc_d[:, 0, 3072:4096])
        nc.sync.dma_start(tc0[:, :1024], xtc_d[:, 0, :1024])
        nc.sync.dma_start(tc0[:, 1024:2048], xtc_d[:, 0, 1024:2048])
        xt_tiles[0] = (
            tc0[:, :2048].bitcast(FP8).rearrange(
                "p (kp ko n) -> p kp ko n", kp=KP8, ko=2),
            tc0[:, 2048:].bitcast(BF).rearrange(
                "p (kb n) -> p kb n", kb=KB16))
        nc.scalar.dma_start(w1eb_sb[:, 0, :512], w1eb_d[:, 0, :512])
        nc.scalar.dma_start(w1eb_sb[:, 0, 512:], w1eb_d[:, 0, 512:])
        nc.scalar.dma_start(w1eb_sb[:, 1], w1eb_d[:, 1])
        nc.scalar.dma_start(w1e8_sb[:, 0, :, :512], w1e8_d[:, 0, :, :512])
        nc.scalar.dma_start(w1e8_sb[:, 0, :, 512:], w1e8_d[:, 0, :, 512:])
        nc.scalar.dma_start(w1e8_sb[:, 1], w1e8_d[:, 1])
        dma_xt(1)
        dma_xt2_pair(0, nc.scalar)
        dma_xt2_pair(1, nc.scalar)
        dma_xt(2)
        dma_xt2_pair(2, nc.sync)
        nc.scalar.dma_start(w2b_sb[:], w2b_d[:])
        nc.scalar.dma_start(idn_sb[:], idn_d[:])
        nc.scalar.dma_start(msk_sb[:], msk_d[:])
        dma_xt(3)
        dma_xt2_pair(3, nc.sync)
        dma_xn(0)
        dma_xn(1)

        # --- own: ownneg128[(il,h), t, b] = -(own'[b,2t+il,h] + b1'[2t+il,h])
        # (primes = x WSCALE).  One N=512 matmul per (t, k) on the
        # j-major layout; interleaved into pair 0's ev stream, one
        # tile behind so its DMAs never stall the PE queue.
        own128 = ownsb.tile([128, MT, BC], BF, tag="own")
        own_ps = {}

        def own_mm(t, kp):
            if t < 0 or t >= MT or kp >= KT // 2:
                return
            if kp == 0:
                own_ps[t] = plps.tile([128, 2, BC], F32, tag="pp",
                                      name="ownp")
            nc.tensor.matmul(
                own_ps[t][:], lhsT=w1o_sb[:, kp, :, t * 128:(t + 1) * 128],
                rhs=xt2_tiles[t][:, kp, :, :],
                start=(kp == 0), stop=(kp == KT // 2 - 1),
                perf_mode=mybir.MatmulPerfMode.DoubleRow,
                skip_group_check=True,
            )

        def own_retire(t):
            # on scalar (activation): out = Copy(in * -1 + b1n) — the
            # vector engine is the busy one during pair 0
            if t >= MT:
                return
            ops = own_ps.pop(t)
            for il in range(2):
                nc.scalar.activation(
                    own128[il * H:(il + 1) * H, t, :],
                    ops[il * H:(il + 1) * H, il, :],
                    mybir.ActivationFunctionType.Identity,
                    bias=b1n_sb[il * H:(il + 1) * H, t, None],
                    scale=-1.0)

        def softmax_cols(scp, wgt, hf):
            # softmax of one 256-column half of the last chunk's scores
            bs = slice(hf * (CB // 2), (hf + 1) * (CB // 2))
            cols = slice(hf * 256, (hf + 1) * 256)
            ex = sm.tile([NINS, CB, NINS], F32, tag="ex")
            ssum = sm.tile([NINS, CB], F32, tag="ssum")
            rinv = sm.tile([NINS, CB], F32, tag="rinv")
            nc.scalar.activation(
                ex[:, bs, :],
                scp[:NINS, :].rearrange("p (b j) -> p b j", j=NINS)[:, bs, :],
                mybir.ActivationFunctionType.Exp)
            nc.vector.tensor_reduce(ssum[:, bs], ex[:, bs, :],
                                    axis=mybir.AxisListType.X,
                                    op=mybir.AluOpType.add)
            nc.vector.reciprocal(rinv[:, bs], ssum[:, bs])
            nc.vector.tensor_tensor(
                wgt.rearrange("p (b j) -> p b j", j=NINS)[:, bs, :],
                ex[:, bs, :],
                rinv[:, bs, None].to_broadcast([NINS, CB // 2, NINS]),
                mybir.AluOpType.mult)

        def do_softmax(scp, halves=1):
            # scores are O(3); safe to exp without max subtraction.
            # only 16 of 128 rows hold scores; the rest are exact zeros
            # (the col-tiled matmuls write zeros there) so everything
            # stays finite and the sel-transpose drops them.
            # halves=2 (last chunk): process column halves separately so
            # pooling of the first half starts while the second runs.
            ex = sm.tile([NINS, CB, NINS], F32, tag="ex")
            ssum = sm.tile([NINS, CB], F32, tag="ssum")
            rinv = sm.tile([NINS, CB], F32, tag="rinv")
            wgt = sm.tile([NINS, CHUNK], BF, tag="wgt")
            hb = CB // halves
            for hf in range(halves):
                bs = slice(hf * hb, (hf + 1) * hb)
                nc.scalar.activation(
                    ex[:, bs, :],
                    scp.rearrange("p (b j) -> p b j", j=NINS)[:, bs, :],
                    mybir.ActivationFunctionType.Exp)
                nc.vector.tensor_reduce(ssum[:, bs], ex[:, bs, :],
                                        axis=mybir.AxisListType.X,
                                        op=mybir.AluOpType.add)
                nc.vector.reciprocal(rinv[:, bs], ssum[:, bs])
                nc.vector.tensor_tensor(
                    wgt.rearrange("p (b j) -> p b j", j=NINS)[:, bs, :],
                    ex[:, bs, :],
                    rinv[:, bs, None].to_broadcast([NINS, hb, NINS]),
                    mybir.AluOpType.mult)
            return wgt

        def emit_wgtT(wgt, rts, tpf=None):
            # borrow one evps ring buffer; bitcast a bf16 view for the
            # transpose outputs ([128, 4, 16] bf16 = 128 f32 bytes).
            # rts selects which 128-column blocks to transpose, so the
            # last chunk can transpose each softmax half as it lands.
            if tpf is None:
                tpf = evps.tile([128, CHUNK], F32, tag="ev")
            tp = tpf[:, :32].bitcast(BF).rearrange("p (r i) -> p r i", i=NINS)
            for rt in rts:
                nc.tensor.transpose(tp[:, rt, :],
                                    wgt[:, rt * 128:(rt + 1) * 128],
                                    idn_sb[:])
            return tpf, tp

        def emit_pool_rt(c, tp, rt):
            bd = bdp.tile([128, 8, NINS], BF, tag="bd")
            nc.vector.tensor_tensor(
                bd[:], tp[:, rt, None, :].to_broadcast([128, 8, NINS]),
                msk_sb.rearrange("p (g i) -> p g i", i=NINS),
                mybir.AluOpType.mult)
            bdf = bd.rearrange("p g i -> p (g i)")
            pp0 = plps.tile([128, 384], F32, tag="pp")
            pp1 = plps.tile([128, 384], F32, tag="pp")
            nc.tensor.matmul(pp0[:], lhsT=bdf, rhs=xn_tiles[c][:, rt, :384],
                             start=True, stop=True, skip_group_check=True)
            nc.tensor.matmul(pp1[:], lhsT=bdf, rhs=xn_tiles[c][:, rt, 384:],
                             start=True, stop=True, skip_group_check=True)
            return pp0, pp1

        def emit_out_rt(c, rt, pp0, pp1, drain=False):
            osb = outp.tile([128, D], BF, tag="osb")
            if drain:
                # tail: consecutive rt pieces reuse the same two PSUM
                # banks, so their copies gate the next pool matmuls —
                # alternate the copy engine per rt so they overlap,
                # and alternate output DMA rings
                if rt % 2:
                    nc.vector.tensor_copy(osb[:, :384], pp0[:])
                    nc.vector.tensor_copy(osb[:, 384:], pp1[:])
                    nc.sync.dma_start(out_d[:, c * 4 + rt, :], osb[:])
                else:
                    nc.scalar.copy(osb[:, :384], pp0[:])
                    nc.scalar.copy(osb[:, 384:], pp1[:])
                    nc.gpsimd.dma_start(out_d[:, c * 4 + rt, :], osb[:])
            else:
                nc.scalar.copy(osb[:, :384], pp0[:])
                nc.scalar.copy(osb[:, 384:], pp1[:])
                nc.sync.dma_start(out_d[:, c * 4 + rt, :], osb[:])

        pqueue = []  # chunks awaiting pooling: {c, wgt, tp, piece}

        SEQ_NORM = [("tp", (0, 1, 2, 3)), ("rt", 0), ("rt", 1),
                    ("rt", 2), ("rt", 3)]
        SEQ_SPLIT = [("tp", (0, 1)), ("rt", 0), ("rt", 1),
                     ("tp", (2, 3)), ("rt", 2), ("rt", 3)]

        def pool_advance(drain=False):
            """Emit the next pooling piece (wgtT, then rt blocks)."""
            if not pqueue:
                return
            st = pqueue[0]
            seq = SEQ_SPLIT if st.get("split") else SEQ_NORM
            kind, arg = seq[st["piece"]]
            if kind == "tp":
                st["tpf"], st["tp"] = emit_wgtT(st["wgt"], arg,
                                                st.get("tpf"))
            else:
                ppa, ppb = emit_pool_rt(st["c"], st["tp"], arg)
                emit_out_rt(st["c"], arg, ppa, ppb, drain=drain)
            st["piece"] += 1
            if st["piece"] == len(seq):
                pqueue.pop(0)

        def emit_scores(hts):
            scp = scps.tile([128, CHUNK], F32, tag="scp")
            for t in range(MT):
                nc.tensor.matmul(
                    scp[:], lhsT=w2b_sb[:, t, :], rhs=hts[t],
                    start=(t == 0), stop=(t == MT - 1),
                    skip_group_check=True,  # rows 16+ all-zero
                )
            return scp

        def ev_mms(cs, t, evs, with_own):
            """ev for tile t of every chunk in cs (shared weight loads);
            pair 0 interleaves own tile t-1 between the k-groups.
            bf16 k-tiles go FIRST: an LDWEIGHTS can overlap a normal
            matmul's stream but not a DoubleRow one, so leading with
            bf16 hides the first DR weight load under the bf16 MMs."""
            ts = [xt_tiles[c] for c in cs]
            ok = iter(range(KT))
            for kb in range(KB16):
                for ci, c in enumerate(cs):
                    nc.tensor.matmul(
                        evs[ci][:],
                        lhsT=w1eb_sb[:, kb, t * 128:(t + 1) * 128],
                        rhs=ts[ci][1][:, kb, :],
                        start=(kb == 0), stop=False,
                        skip_group_check=True,
                    )
                if with_own:
                    own_mm(t - 1, next(ok))
            for kp in range(KP8):
                for ci, c in enumerate(cs):
                    nc.tensor.matmul(
                        evs[ci][:],
                        lhsT=w1e8_sb[:, kp, :, t * 128:(t + 1) * 128],
                        rhs=ts[ci][0][:, kp, :, :],
                        start=False, stop=(kp == KP8 - 1),
                        perf_mode=mybir.MatmulPerfMode.DoubleRow,
                        skip_group_check=True,
                    )
                if with_own:
                    own_mm(t - 1, next(ok))
                    own_mm(t - 1, next(ok))

        def emit_h(c, t, evp):
            h_t = hp.tile([128, CB, NINS], BF, tag="h")
            nc.vector.tensor_tensor(
                h_t[:], evp.rearrange("p (b j) -> p b j", j=NINS),
                own128[:, t, c * CB:(c + 1) * CB, None]
                .to_broadcast([128, CB, NINS]),
                mybir.AluOpType.max)
            return h_t.rearrange("p b j -> p (b j)")

        def emit_group(cs, with_own=False, last=False):
            """Process chunks cs (1 or 2) through ev/h/scores/softmax,
            while draining the pooling of previous chunks."""
            hts = {c: [] for c in cs}
            lag = 1 if with_own else 0
            evps_t = {}
            for t in range(MT + lag):
                if t < MT:
                    evps_t[t] = [evps.tile([128, CHUNK], F32, tag="ev",
                                           name="evt")
                                 for _ in cs]
                    ev_mms(cs, t, evps_t[t], with_own)
                elif with_own:
                    # own tile 7 has no ev matmuls left to ride on
                    for k in range(KT):
                        own_mm(MT - 1, k)
                th = t - lag
                if 0 <= th < MT:
                    if with_own:
                        own_retire(th)
                    for ci, c in enumerate(cs):
                        hts[c].append(emit_h(c, th, evps_t[th][ci]))
                        # skip the first slot: gives the previous
                        # chunk's softmax time to finish before wgtT
                        if th >= 1:
                            pool_advance()
                    del evps_t[th]
                if t == 4:
                    for c in cs:
                        dma_xn(c + 2)
            if last:
                # final chunk: scores and softmax in column halves so
                # the pooling drain starts while half B is still going
                c = cs[0]
                scp = scps.tile([128, CHUNK], F32, tag="scp")
                wgt = sm.tile([NINS, CHUNK], BF, tag="wgt")
                for hf in range(2):
                    cols = slice(hf * 256, (hf + 1) * 256)
                    for t in range(MT):
                        nc.tensor.matmul(
                            scp[:, cols], lhsT=w2b_sb[:, t, :],
                            rhs=hts[c][t][:, cols],
                            start=(t == 0), stop=(t == MT - 1),
                            skip_group_check=True)
                    softmax_cols(scp, wgt, hf)
                pqueue.append({"c": c, "wgt": wgt, "tp": None,
                               "piece": 0, "split": True})
                return
            scps_c = [emit_scores(hts[c]) for c in cs]
            # pooling not covered by the h slots drains AFTER scores:
            # it has slack, while scores gate the softmax/pool chain
            while pqueue:
                pool_advance()
            for c in cs:
                dma_xt(c + 4)
            for ci, c in enumerate(cs):
                pqueue.append({"c": c,
                               "wgt": do_softmax(scps_c[ci][:NINS, :]),
                               "tp": None, "piece": 0})

        emit_group([0, 1], with_own=True)
        emit_group([2, 3])
        emit_group([4, 5])
        emit_group([6])
        emit_group([7], last=True)

        # drain the last chunk's pooling
        while pqueue:
            pool_advance(drain=True)

    nc.compile()
    return nc


def host_prep(W1, b1, W2):
    """Build the replicated parameter tensors (numpy)."""
    W1 = np.asarray(W1, dtype=np.float32)
    b1 = np.asarray(b1, dtype=np.float32)
    W2 = np.asarray(W2, dtype=np.float32)
    W1o, W1e = W1[:, :D, :] * WSCALE, W1[:, D:, :] * WSCALE

    def to_cols(w):  # [16, 768, 64] -> [768, 1024] (cols i*64+h)
        return np.ascontiguousarray(
            w.transpose(1, 0, 2).reshape(D, NINS * H))

    we = to_cols(W1e)   # [768, 1024], pre-scaled
    # fp8 DoubleRow part: d-tiles 0..3 -> [128, KP8, 2, 1024]
    w1e8 = np.ascontiguousarray(
        we[:512].reshape(KP8, 2, 128, NINS * H)
        .transpose(2, 0, 1, 3)).astype(FP8_NP)
    # bf16 part: d-tiles 4..5 -> [128, KB16, 1024]
    w1eb = np.ascontiguousarray(
        we[512:].reshape(KB16, 128, NINS * H)
        .transpose(1, 0, 2)).astype(BF_NP)
    w1o8 = np.ascontiguousarray(
        to_cols(W1o).reshape(KT // 2, 2, 128, NINS * H)
        .transpose(2, 0, 1, 3)).astype(FP8_NP)

    w2b = np.zeros((128, MT, 128), dtype=np.float32)
    b1n = np.zeros((128, MT), dtype=np.float32)
    for t in range(MT):
        for il in range(2):
            i = 2 * t + il
            w2b[il * H:(il + 1) * H, t, i] = W2[i] / WSCALE
            b1n[il * H:(il + 1) * H, t] = -b1[i] * WSCALE
    p = np.arange(128)
    msk = (p[:, None] // NINS == p[None, :] // NINS).astype(BF_NP)
    idn = np.eye(16, dtype=np.float32).astype(BF_NP)
    return dict(w1e8=w1e8, w1eb=w1eb, w1o8=w1o8, w2b=w2b.astype(BF_NP),
                b1n=b1n, msk=msk, idn=idn)


def get_nc():
    global _CACHED_NC
    if _CACHED_NC is None:
        _CACHED_NC = build_nc()
    return _CACHED_NC


def make_in_maps(inputs, W1, b1, W2):
    consts = host_prep(W1, b1, W2)
    inputs = np.asarray(inputs, dtype=np.float32)
    in_maps = []
    for core in range(NCORES):
        shard = np.ascontiguousarray(
            inputs[core * BC:(core + 1) * BC].reshape(R, D))
        m = dict(consts)
        # natural rows, blocked: xn[p, blk, :] = x[blk*128+p, :]
        m["xn"] = np.ascontiguousarray(
            shard.reshape(NBLK, 128, D).transpose(1, 0, 2)).astype(BF_NP)
        st = shard.T  # [768, 4096]
        # ev chunk data packed as bytes: fp8 DoubleRow part (d-tiles
        # 0..3, (kp, ko) interleave) then bf16 part (d-tiles 4..5)
        xt8 = np.ascontiguousarray(
            st[:512].reshape(KP8, 2, 128, NCH, CHUNK)
            .transpose(2, 3, 0, 1, 4)).astype(FP8_NP)
        xtb = np.ascontiguousarray(
            st[512:].reshape(KB16, 128, NCH, CHUNK)
            .transpose(1, 2, 0, 3)).astype(BF_NP)
        m["xtc"] = np.concatenate(
            [xt8.reshape(128, NCH, 2048).view(np.uint8),
             xtb.reshape(128, NCH, 1024).view(np.uint8)], axis=2)
        # j-major transpose for own: rows (j, b); xt2[p, t, k, col]
        x2 = shard.reshape(BC, NINS, D).transpose(1, 0, 2).reshape(R, D)
        m["xt2"] = np.ascontiguousarray(
            x2.T.reshape(KT // 2, 2, 128, MT, CHUNK)
            .transpose(2, 3, 0, 1, 4)).astype(FP8_NP)
        in_maps.append(m)
    return in_maps


def kernel(inputs, W1, b1, W2, b2, trace=False):
    """Full-input entry point: shards over 8 cores, returns full output."""
    global LAST_RESULTS
    nc = get_nc()
    in_maps = make_in_maps(inputs, W1, b1, W2)
    res = bass_utils.run_bass_kernel_spmd(
        nc, in_maps, core_ids=list(range(NCORES)), trace=trace)
    LAST_RESULTS = res
    out = np.concatenate(
        [np.asarray(r["out"]).astype(np.float32).transpose(1, 0, 2)
         .reshape(BC, NINS, D)
         for r in res.results],
        axis=0)
    return out


if __name__ == "__main__":
    if "--build" in sys.argv:
        get_nc()
        print("build OK")
